# revision 16
# baseline (speedup 1.0000x reference)
"""Paged GQA attention (diffusion block-causal) on 8 TRN2 NeuronCores.

Problem: B=8 seqs x LQ=128 new tokens, 32 q heads / 8 kv heads, head_dim 128,
ctx_len=2048 cached tokens per seq (paged KV cache, 16-token pages), plus the
128 new tokens; block-causal mask (block 32) over the new-token region.

Sharding: one sequence per NeuronCore (8 seqs -> 8 cores), no collectives.

Per-core device kernel, per kv-head group g (4 q heads share a kv head):
  scoresT[k, q4] = K_g^T-tile.T @ Q_g          (bf16 matmul, N=512)
  probsT = exp(scoresT * scale)                split across TWO engines:
      ACT chunks:  scalar.activation Exp (exact)
      DVE chunks:  Schraudolph bit-trick: int16(round(s*C + B)) bitcast bf16
                   (C = scale*log2e*128, B = 127*128 + c_corr) ~2-3% elem err
                   on 6/17 tiles -> ~1e-2 output rel err (gate 2e-2)
  probsT[new] *= blockmask                     (Pool/GpSimd, 0/1 multiplicative)
  outU[q, d+1] += probsT_head.T @ [V_g | 1]    (bf16 matmul; last col = sum)
  out[q, d] = outU[:, :d] * (1 / outU[:, d])   (DVE reciprocal + tensor_scalar)

DMA triggers ride the GpSimd queue (cheap: ~25ns vs ~600ns on SP).
Host side: gather pages, build K^T / Q^T / V-augmented layouts, shard, gather.
"""

import sys

if '/opt/trn_rl_repo' not in sys.path:
    sys.path.insert(0, '/opt/trn_rl_repo')

import math

import ml_dtypes
import numpy as np

B = 8
LQ = 128
NH = 32
NKV = 8
GROUP = NH // NKV  # 4
HD = 128
PAGE = 16
CTX = 2048
K_TOT = CTX + LQ          # 2176
KT_TILES = K_TOT // 128   # 17
QG = GROUP * LQ           # 512 (4 heads x 128 queries)
SCALE = 1.0 / math.sqrt(HD)

# ---- tuning flags ----
SCHRAUD_CHUNKS = (2, 4)   # chunk indices exp'd on DVE via Schraudolph
SCHRAUD_C = -8.0          # bias correction (tuned in numpy sim)
MASK_ON_POOL = True       # block-causal mask multiply on GpSimd
PIPE = 3                  # PV trails QK/exp by this many chunks
IN_BUFS = 3               # kt/va/qt tile-pool depth (prefetch 2 groups ahead)
WARM_MMS = 2              # dummy matmuls to ramp the PE p-state during DMA wait

SCH_C7 = SCALE * math.log2(math.e) * 128.0
SCH_B = 127.0 * 128.0 + SCHRAUD_C

_CACHE = {}


def _build_nc():
    import concourse.bass as bass
    import concourse.mybir as mybir
    from concourse.tile import TileContext
    from concourse.vector_clock import ScopedClock

    class TileContextP(TileContext):
        """TileContext adapted to this walrus build, which only supports ONE
        sync-wait per instruction: extra waits are hoisted onto same-engine
        NoOps emitted immediately before the instruction."""

        def _commit_instruction(self, inst, lazy_reg_writes=True):
            si = getattr(inst, "sync_info", None)
            eng = getattr(inst, "engine", None)
            if si is not None and eng is not None:
                waits = list(si.on_wait or [])
                if len(waits) > 1:
                    for w in waits[:-1]:
                        nop = mybir.InstNoOp(
                            name=self.nc.get_next_instruction_name(),
                            sync_info=mybir.SyncInfo(on_wait=[w], on_update=[]),
                            bass_nofuse=True,
                            engine=eng,
                        )
                        super()._commit_instruction(nop, lazy_reg_writes=False)
                    si.on_wait = [waits[-1]]
            return super()._commit_instruction(inst, lazy_reg_writes)

        def _drain_and_barrier(self, tick_clock, wait_clock):
            nc = self.nc
            drain_inst = nc.sync.drain()
            wait_clock.add_sem_waits(
                drain_inst.ins, ScopedClock({None: tick_clock.global_clock})
            )
            si = drain_inst.ins.sync_info
            waits = list(si.on_wait or []) if si is not None else []
            if len(waits) > 1:
                si.on_wait = [waits[0]]
                # distribute remaining waits round-robin across engines so
                # they resolve in parallel rather than serially on SP
                engs = [nc.vector, nc.scalar, nc.tensor, nc.gpsimd, nc.sync]
                for j, w in enumerate(waits[1:]):
                    d = engs[j % len(engs)].drain()
                    d.ins.sync_info = mybir.SyncInfo(on_wait=[w], on_update=[])
            nc.all_engine_barrier(sem_only=True)
            assert self.sems is not None
            popped = nc._tile_sem_poison_stack.pop()
            assert popped is self._sem_poison
            nc.clear_and_free_semaphores(list(self.sems.allocated().values()))
            nc.all_engine_barrier(sem_only=True)

    f32 = mybir.dt.float32
    bf16 = mybir.dt.bfloat16
    i16 = mybir.dt.int16

    nc = bass.Bass("TRN2")
    qt = nc.dram_tensor("qt", [NKV, HD, QG], bf16, kind="ExternalInput")
    kt = nc.dram_tensor("kt", [NKV, HD, K_TOT], bf16, kind="ExternalInput")
    va = nc.dram_tensor("va", [NKV, 128, KT_TILES, HD + 1], bf16,
                        kind="ExternalInput")
    mk = nc.dram_tensor("mk", [128, QG], bf16, kind="ExternalInput")
    o = nc.dram_tensor("o", [LQ, NH * HD], bf16, kind="ExternalOutput")

    # k-tiles are exp'd in triples: one [128, 3*QG] PSUM chunk spans three
    # banks, cutting the per-instruction fixed cost. 17 tiles -> 5 triples +
    # 1 pair. PSUM: scores 3 banks x 2 bufs + 2 packed accumulator banks = 8.
    # The masked pair (tiles 15,16) is processed FIRST so the Pool mask
    # multiply overlaps the remaining chunks instead of gating the group's
    # last PV; PSUM accumulation order is irrelevant (sum).
    chunks = [(15, 2)] + [(3 * i, 3) for i in range(5)]
    first_tile, last_tile = 15, 14

    with TileContextP(nc) as tc:
        maske = nc.gpsimd if MASK_ON_POOL else nc.vector
        with (
            tc.tile_pool(name="kp", bufs=IN_BUFS) as kp,
            tc.tile_pool(name="vp", bufs=IN_BUFS) as vp,
            tc.tile_pool(name="qp", bufs=IN_BUFS) as qp,
            tc.tile_pool(name="mp", bufs=1) as mp,
            tc.tile_pool(name="pp", bufs=5) as pp,
            tc.tile_pool(name="rp", bufs=4) as rp,
            tc.tile_pool(name="ob", bufs=2) as ob,
            tc.tile_pool(name="sp", bufs=2, space="PSUM") as sp,
            tc.tile_pool(name="op", bufs=2, space="PSUM") as op,
        ):
            # warm the ACT exp table while the first DMAs are in flight
            warm = mp.tile([128, 1], mybir.dt.float32, name="warm")
            nc.scalar.memzero(warm)
            nc.scalar.activation(warm, warm,
                                 mybir.ActivationFunctionType.Exp)

            mask_sb = mp.tile([128, QG], bf16)

            # ramp the PE p-state (cold: 0.65 GHz -> 2.4 GHz after ~3us of
            # work) with dummy matmuls on zeroed SBUF while DMAs land
            if WARM_MMS:
                z_l = mp.tile([128, 128], bf16, name="warm_l")
                z_r = mp.tile([128, QG], bf16, name="warm_r")
                nc.gpsimd.memzero(z_l)
                nc.gpsimd.memzero(z_r)
                warm_ps = sp.tile([128, 3 * QG], mybir.dt.float32, tag="s",
                                  name="warm_ps")
                for _ in range(WARM_MMS):
                    nc.tensor.matmul(warm_ps[:, :QG], lhsT=z_l, rhs=z_r,
                                     start=True, stop=True)

            # Flat software pipeline over (group, chunk): each chunk's PV
            # matmuls are emitted AFTER later chunks' QK matmuls so the
            # in-order PE stream never head-of-line blocks on exp results or
            # on the previous group's accumulator release.
            accs_of = {}
            qt_of, kt_of, va_of = {}, {}, {}

            def load_group(g, engs=None):
                # hardware-DGE trigger engines; prologue groups parallelize
                # across idle queues, steady state rides SP (625ns/trigger)
                if engs is None:
                    engs = [nc.sync] * 5
                qt_sb = qp.tile([HD, QG], bf16, tag="qt", name=f"qt{g}")
                engs[0].dma_start(out=qt_sb, in_=qt[g])
                kt_sb = kp.tile([HD, K_TOT], bf16, tag="kt", name=f"kt{g}")
                # split so the first-processed chunks' K columns land early:
                # masked tail pair (tiles 15,16) first, then tiles 0-5, 6-14
                engs[1].dma_start(out=kt_sb[:, 15 * 128:],
                                  in_=kt[g][:, 15 * 128:])
                engs[2].dma_start(out=kt_sb[:, :6 * 128],
                                  in_=kt[g][:, :6 * 128])
                engs[3].dma_start(out=kt_sb[:, 6 * 128:15 * 128],
                                  in_=kt[g][:, 6 * 128:15 * 128])
                va_sb = vp.tile([128, KT_TILES, HD + 1], bf16, tag="va",
                                name=f"va{g}")
                engs[4].dma_start(out=va_sb, in_=va[g])
                kt_of[g], va_of[g], qt_of[g] = kt_sb, va_sb, qt_sb
                # two heads' [q, d+1] accumulators packed per PSUM bank
                accs_of[g] = [
                    op.tile([LQ, 2, HD + 1], mybir.dt.float32, tag="acc",
                            name=f"acc_{g}_{p}")
                    for p in range(GROUP // 2)
                ]

            def emit_qk_exp(g, ci, t0, width):
                s_ps = sp.tile([128, 3 * QG], mybir.dt.float32, tag="s",
                               name=f"s_{g}_{t0}")
                for tt in range(width):
                    nc.tensor.matmul(
                        s_ps[:, tt * QG:(tt + 1) * QG],
                        lhsT=kt_of[g][:, (t0 + tt) * 128:(t0 + tt + 1) * 128],
                        rhs=qt_of[g],
                        start=True,
                        stop=True,
                    )
                p_sb = pp.tile([128, 3 * QG], bf16, tag="p",
                               name=f"p_{g}_{t0}")
                if ci in SCHRAUD_CHUNKS:
                    # Schraudolph fast-exp on DVE: the int16 affine result IS
                    # the bf16 bit pattern of ~exp(s*scale)
                    nc.vector.tensor_scalar(
                        p_sb[:, :width * QG].bitcast(i16),
                        s_ps[:, :width * QG],
                        SCH_C7, SCH_B,
                        mybir.AluOpType.mult, mybir.AluOpType.add,
                    )
                else:
                    nc.scalar.activation(
                        p_sb[:, :width * QG], s_ps[:, :width * QG],
                        mybir.ActivationFunctionType.Exp, scale=SCALE,
                    )
                if t0 + width == KT_TILES:
                    # new-token tile: multiplicative block-causal mask applied
                    # post-exp (keeps ACT's critical path mask-free)
                    lo = (width - 1) * QG
                    maske.tensor_mul(
                        p_sb[:, lo:lo + QG], p_sb[:, lo:lo + QG], mask_sb)
                return p_sb

            def emit_pv(g, ci, t0, width, p_sb):
                for tt in range(width):
                    t = t0 + tt
                    for h in range(GROUP):
                        # start=True clears has_written for the WHOLE bank, so
                        # only the first head sharing the bank issues it; the
                        # second head's first-tile write lands on
                        # has_written=0 and overwrites rather than accumulates.
                        nc.tensor.matmul(
                            accs_of[g][h // 2][:, h % 2, :],
                            lhsT=p_sb[:, tt * QG + h * LQ:
                                      tt * QG + (h + 1) * LQ],
                            rhs=va_of[g][:, t, :],
                            start=(t == first_tile and h % 2 == 0),
                            stop=(t == last_tile),
                            skip_group_check=True,
                        )

            def emit_normalize(g):
                o_sb = ob.tile([128, GROUP * HD], bf16, tag="osb",
                               name=f"osb{g}")
                for p in range(GROUP // 2):
                    acc = accs_of[g][p]
                    rec = rp.tile([LQ, 2], mybir.dt.float32, tag="rec",
                                  name=f"rec_{g}_{p}")
                    nc.vector.reciprocal(rec, acc[:, :, HD:HD + 1])
                    for hh in range(2):
                        h = 2 * p + hh
                        nc.vector.tensor_scalar_mul(
                            o_sb[:, h * HD:(h + 1) * HD],
                            acc[:, hh, 0:HD],
                            rec[:, hh:hh + 1],
                        )
                nc.sync.dma_start(
                    out=o[:, g * GROUP * HD:(g + 1) * GROUP * HD], in_=o_sb)

            work = [(g, ci, t0, w) for g in range(NKV)
                    for ci, (t0, w) in enumerate(chunks)]
            load_group(0, engs=[nc.sync, nc.scalar, nc.sync, nc.scalar,
                                nc.sync])
            nc.scalar.dma_start(out=mask_sb, in_=mk[:, :])
            load_group(1, engs=[nc.scalar, nc.sync, nc.scalar, nc.sync,
                                nc.scalar])
            pending = []  # (g, ci, t0, width, p_sb) awaiting PV emission
            for g, ci, t0, w in work:
                if t0 == 0 and g + 2 < NKV:
                    load_group(g + 2)
                p_sb = emit_qk_exp(g, ci, t0, w)
                pending.append((g, ci, t0, w, p_sb))
                if len(pending) > PIPE:
                    ent = pending.pop(0)
                    emit_pv(*ent)
                    if ent[1] == len(chunks) - 1:
                        emit_normalize(ent[0])
            for ent in pending:
                emit_pv(*ent)
                if ent[1] == len(chunks) - 1:
                    emit_normalize(ent[0])
    return nc


def _prep_inputs(q, k, v, k_cache, v_cache, page_tables, ctx_len, block_size):
    ctx = int(ctx_len)
    bs = int(block_size)
    assert ctx == CTX, f"kernel compiled for ctx_len={CTX}, got {ctx}"
    npages = ctx // PAGE

    q = np.asarray(q, np.float32).reshape(B, LQ, NH, HD)
    k = np.asarray(k, np.float32).reshape(B, LQ, NKV, HD)
    v = np.asarray(v, np.float32).reshape(B, LQ, NKV, HD)
    k_cache = np.asarray(k_cache, np.float32)
    v_cache = np.asarray(v_cache, np.float32)
    pt = np.asarray(page_tables).astype(np.int64)[:, :npages]

    # paged gather: [B, ctx, NKV, HD]
    k_ctx = k_cache[pt].reshape(B, ctx, NKV, HD)
    v_ctx = v_cache[pt].reshape(B, ctx, NKV, HD)
    k_full = np.concatenate([k_ctx, k], axis=1)   # [B, K_TOT, NKV, HD]
    v_full = np.concatenate([v_ctx, v], axis=1)

    # K^T per core: [NKV, HD, K_TOT], bf16
    kt = np.ascontiguousarray(
        k_full.transpose(0, 2, 3, 1)).astype(ml_dtypes.bfloat16)
    # V augmented with ones column, bf16: [B, NKV, K_TOT, HD+1]
    v_t = v_full.transpose(0, 2, 1, 3)            # [B, NKV, K_TOT, HD]
    va = np.empty((B, NKV, K_TOT, HD + 1), np.float32)
    va[..., :HD] = v_t
    va[..., HD] = 1.0
    va = va.astype(ml_dtypes.bfloat16).reshape(B, NKV, KT_TILES, 128, HD + 1)
    # swizzle so each SBUF partition line is one contiguous DMA burst
    va = np.ascontiguousarray(va.transpose(0, 1, 3, 2, 4))
    # Q^T per group: [B, NKV, HD, GROUP*LQ]
    qh = q.transpose(0, 2, 3, 1).reshape(B, NKV, GROUP, HD, LQ)
    qt = np.ascontiguousarray(qh.transpose(0, 1, 3, 2, 4)).reshape(
        B, NKV, HD, QG).astype(ml_dtypes.bfloat16)

    # multiplicative 0/1 mask on the new-token k-tile, scoresT coords [k, q4]
    kj = np.arange(128)
    qi = np.arange(LQ)
    allowed = (kj[:, None] // bs) <= (qi[None, :] // bs)   # [128, LQ]
    mrow = allowed.astype(np.float32)
    mask = np.tile(mrow, (1, GROUP)).astype(ml_dtypes.bfloat16)  # [128, QG]

    in_maps = []
    for b in range(B):
        in_maps.append({
            "qt": qt[b],
            "kt": kt[b],
            "va": va[b],
            "mk": mask,
        })
    return in_maps


def _run(inputs, trace=False):
    from concourse.bass_utils import run_bass_kernel_spmd

    if "nc" not in _CACHE:
        _CACHE["nc"] = _build_nc()
    nc = _CACHE["nc"]
    in_maps = _prep_inputs(**inputs)
    try:
        res = run_bass_kernel_spmd(
            nc, in_maps, core_ids=list(range(B)), trace=trace,
        )
    except Exception:
        # transient NRT device errors have been observed once after heavy
        # compile churn; one retry on a quiesced device is reliable
        import time
        time.sleep(2)
        res = run_bass_kernel_spmd(
            nc, in_maps, core_ids=list(range(B)), trace=trace,
        )
    out = np.empty((B * LQ, NH * HD), np.float32)
    for b in range(B):
        out[b * LQ:(b + 1) * LQ] = np.asarray(
            res.results[b]["o"]).astype(np.float32)
    return out, res


def kernel(**inputs):
    out, _ = _run(inputs, trace=False)
    return out


# revision 19
# speedup vs baseline: 1.2209x; 1.2209x over previous
"""Paged GQA attention (diffusion block-causal) on 8 TRN2 NeuronCores.

Problem: B=8 seqs x LQ=128 new tokens, 32 q heads / 8 kv heads, head_dim 128,
ctx_len=2048 cached tokens per seq (paged KV cache, 16-token pages), plus the
128 new tokens; block-causal mask (block 32) over the new-token region.

Sharding: one sequence per NeuronCore (8 seqs -> 8 cores), no collectives.

Per-core device kernel, per kv-head group g (4 q heads share a kv head):
  scoresT[k, q4] = K_g^T-tile.T @ Q_g          (bf16 matmul, N=512)
  probsT = exp(scoresT * scale)                split across TWO engines:
      ACT chunks:  scalar.activation Exp (exact)
      DVE chunks:  Schraudolph bit-trick: int16(round(s*C + B)) bitcast bf16
                   (C = scale*log2e*128, B = 127*128 + c_corr) ~2-3% elem err
                   on 6/17 tiles -> ~1e-2 output rel err (gate 2e-2)
  probsT[new] *= blockmask                     (Pool/GpSimd, 0/1 multiplicative)
  outU[q, d+1] += probsT_head.T @ [V_g | 1]    (bf16 matmul; last col = sum)
  out[q, d] = outU[:, :d] * (1 / outU[:, d])   (DVE reciprocal + tensor_scalar)

DMA triggers ride the GpSimd queue (cheap: ~25ns vs ~600ns on SP).
Host side: gather pages, build K^T / Q^T / V-augmented layouts, shard, gather.
"""

import sys

if '/opt/trn_rl_repo' not in sys.path:
    sys.path.insert(0, '/opt/trn_rl_repo')

import math

import ml_dtypes
import numpy as np

B = 8
LQ = 128
NH = 32
NKV = 8
GROUP = NH // NKV  # 4
HD = 128
PAGE = 16
CTX = 2048
K_TOT = CTX + LQ          # 2176
KT_TILES = K_TOT // 128   # 17
QG = GROUP * LQ           # 512 (4 heads x 128 queries)
SCALE = 1.0 / math.sqrt(HD)

# ---- tuning flags ----
SCHRAUD_CHUNKS = (1, 3)   # chunk indices exp'd on DVE via Schraudolph
SCHRAUD_C = -8.0          # bias correction (tuned in numpy sim)
MASK_ON_POOL = True       # block-causal mask multiply on GpSimd
PIPE = 3                  # PV trails QK/exp by this many chunks
IN_BUFS = 3               # kt/va/qt tile-pool depth (prefetch 2 groups ahead)
WARM_MMS = 2              # dummy matmuls to ramp the PE p-state during DMA wait

SCH_C7 = SCALE * math.log2(math.e) * 128.0
SCH_B = 127.0 * 128.0 + SCHRAUD_C

_CACHE = {}


def _build_nc():
    import concourse.bass as bass
    import concourse.mybir as mybir
    from concourse.tile import TileContext
    from concourse.vector_clock import ScopedClock

    class TileContextP(TileContext):
        """TileContext adapted to this walrus build, which only supports ONE
        sync-wait per instruction: extra waits are hoisted onto same-engine
        NoOps emitted immediately before the instruction."""

        def _commit_instruction(self, inst, lazy_reg_writes=True):
            si = getattr(inst, "sync_info", None)
            eng = getattr(inst, "engine", None)
            if si is not None and eng is not None:
                waits = list(si.on_wait or [])
                if len(waits) > 1:
                    for w in waits[:-1]:
                        nop = mybir.InstNoOp(
                            name=self.nc.get_next_instruction_name(),
                            sync_info=mybir.SyncInfo(on_wait=[w], on_update=[]),
                            bass_nofuse=True,
                            engine=eng,
                        )
                        super()._commit_instruction(nop, lazy_reg_writes=False)
                    si.on_wait = [waits[-1]]
            return super()._commit_instruction(inst, lazy_reg_writes)

        def _drain_and_barrier(self, tick_clock, wait_clock):
            nc = self.nc
            drain_inst = nc.sync.drain()
            wait_clock.add_sem_waits(
                drain_inst.ins, ScopedClock({None: tick_clock.global_clock})
            )
            si = drain_inst.ins.sync_info
            waits = list(si.on_wait or []) if si is not None else []
            if len(waits) > 1:
                si.on_wait = [waits[0]]
                # distribute remaining waits round-robin across engines so
                # they resolve in parallel rather than serially on SP
                engs = [nc.vector, nc.scalar, nc.tensor, nc.gpsimd, nc.sync]
                for j, w in enumerate(waits[1:]):
                    d = engs[j % len(engs)].drain()
                    d.ins.sync_info = mybir.SyncInfo(on_wait=[w], on_update=[])
            nc.all_engine_barrier(sem_only=True)
            assert self.sems is not None
            popped = nc._tile_sem_poison_stack.pop()
            assert popped is self._sem_poison
            nc.clear_and_free_semaphores(list(self.sems.allocated().values()))
            nc.all_engine_barrier(sem_only=True)

    f32 = mybir.dt.float32
    bf16 = mybir.dt.bfloat16
    i16 = mybir.dt.int16

    nc = bass.Bass("TRN2")
    qt = nc.dram_tensor("qt", [NKV, HD, QG], bf16, kind="ExternalInput")
    kt = nc.dram_tensor("kt", [NKV, HD, K_TOT], bf16, kind="ExternalInput")
    va = nc.dram_tensor("va", [NKV, 128, KT_TILES, HD + 1], bf16,
                        kind="ExternalInput")
    mk = nc.dram_tensor("mk", [128, QG], bf16, kind="ExternalInput")
    o = nc.dram_tensor("o", [LQ, NH * HD], bf16, kind="ExternalOutput")

    # k-tiles are exp'd in triples: one [128, 3*QG] PSUM chunk spans three
    # banks, cutting the per-instruction fixed cost. 17 tiles -> 5 triples +
    # 1 pair. PSUM: scores 3 banks x 2 bufs + 2 packed accumulator banks = 8.
    chunks = [(3 * i, 3) for i in range(5)] + [(15, 2)]
    first_tile, last_tile = 0, KT_TILES - 1

    with TileContextP(nc) as tc:
        maske = nc.gpsimd if MASK_ON_POOL else nc.vector
        with (
            tc.tile_pool(name="kp", bufs=IN_BUFS) as kp,
            tc.tile_pool(name="vp", bufs=IN_BUFS) as vp,
            tc.tile_pool(name="qp", bufs=IN_BUFS) as qp,
            tc.tile_pool(name="mp", bufs=1) as mp,
            tc.tile_pool(name="pp", bufs=5) as pp,
            tc.tile_pool(name="rp", bufs=4) as rp,
            tc.tile_pool(name="ob", bufs=2) as ob,
            tc.tile_pool(name="sp", bufs=2, space="PSUM") as sp,
            tc.tile_pool(name="op", bufs=2, space="PSUM") as op,
        ):
            # warm the ACT exp table while the first DMAs are in flight
            warm = mp.tile([128, 1], mybir.dt.float32, name="warm")
            nc.scalar.memzero(warm)
            nc.scalar.activation(warm, warm,
                                 mybir.ActivationFunctionType.Exp)

            mask_sb = mp.tile([128, QG], bf16)

            # ramp the PE p-state (cold: 0.65 GHz -> 2.4 GHz after ~3us of
            # work) with dummy matmuls on zeroed SBUF while DMAs land
            if WARM_MMS:
                z_l = mp.tile([128, 128], bf16, name="warm_l")
                z_r = mp.tile([128, QG], bf16, name="warm_r")
                nc.gpsimd.memzero(z_l)
                nc.gpsimd.memzero(z_r)
                warm_ps = sp.tile([128, 3 * QG], mybir.dt.float32, tag="s",
                                  name="warm_ps")
                for _ in range(WARM_MMS):
                    nc.tensor.matmul(warm_ps[:, :QG], lhsT=z_l, rhs=z_r,
                                     start=True, stop=True)

            # Flat software pipeline over (group, chunk): each chunk's PV
            # matmuls are emitted AFTER later chunks' QK matmuls so the
            # in-order PE stream never head-of-line blocks on exp results or
            # on the previous group's accumulator release.
            accs_of = {}
            qt_of, kt_of, va_of = {}, {}, {}

            def load_group(g, engs=None):
                # hardware-DGE trigger engines; prologue groups parallelize
                # across idle queues, steady state rides SP (625ns/trigger)
                if engs is None:
                    engs = [nc.sync] * 5
                qt_sb = qp.tile([HD, QG], bf16, tag="qt", name=f"qt{g}")
                engs[0].dma_start(out=qt_sb, in_=qt[g])
                kt_sb = kp.tile([HD, K_TOT], bf16, tag="kt", name=f"kt{g}")
                # split so the first chunks' K columns land early
                engs[1].dma_start(out=kt_sb[:, :3 * 128],
                                  in_=kt[g][:, :3 * 128])
                engs[2].dma_start(out=kt_sb[:, 3 * 128:6 * 128],
                                  in_=kt[g][:, 3 * 128:6 * 128])
                engs[3].dma_start(out=kt_sb[:, 6 * 128:],
                                  in_=kt[g][:, 6 * 128:])
                va_sb = vp.tile([128, KT_TILES, HD + 1], bf16, tag="va",
                                name=f"va{g}")
                engs[4].dma_start(out=va_sb, in_=va[g])
                kt_of[g], va_of[g], qt_of[g] = kt_sb, va_sb, qt_sb
                # two heads' [q, d+1] accumulators packed per PSUM bank
                accs_of[g] = [
                    op.tile([LQ, 2, HD + 1], mybir.dt.float32, tag="acc",
                            name=f"acc_{g}_{p}")
                    for p in range(GROUP // 2)
                ]

            def emit_qk_exp(g, ci, t0, width):
                s_ps = sp.tile([128, 3 * QG], mybir.dt.float32, tag="s",
                               name=f"s_{g}_{t0}")
                for tt in range(width):
                    nc.tensor.matmul(
                        s_ps[:, tt * QG:(tt + 1) * QG],
                        lhsT=kt_of[g][:, (t0 + tt) * 128:(t0 + tt + 1) * 128],
                        rhs=qt_of[g],
                        start=True,
                        stop=True,
                    )
                p_sb = pp.tile([128, 3 * QG], bf16, tag="p",
                               name=f"p_{g}_{t0}")
                if ci in SCHRAUD_CHUNKS:
                    # Schraudolph fast-exp on DVE: the int16 affine result IS
                    # the bf16 bit pattern of ~exp(s*scale)
                    nc.vector.tensor_scalar(
                        p_sb[:, :width * QG].bitcast(i16),
                        s_ps[:, :width * QG],
                        SCH_C7, SCH_B,
                        mybir.AluOpType.mult, mybir.AluOpType.add,
                    )
                else:
                    nc.scalar.activation(
                        p_sb[:, :width * QG], s_ps[:, :width * QG],
                        mybir.ActivationFunctionType.Exp, scale=SCALE,
                    )
                if t0 + width == KT_TILES:
                    # new-token tile: multiplicative block-causal mask applied
                    # post-exp (keeps ACT's critical path mask-free)
                    lo = (width - 1) * QG
                    maske.tensor_mul(
                        p_sb[:, lo:lo + QG], p_sb[:, lo:lo + QG], mask_sb)
                return p_sb

            def emit_pv(g, ci, t0, width, p_sb):
                for tt in range(width):
                    t = t0 + tt
                    for h in range(GROUP):
                        # start=True clears has_written for the WHOLE bank, so
                        # only the first head sharing the bank issues it; the
                        # second head's first-tile write lands on
                        # has_written=0 and overwrites rather than accumulates.
                        nc.tensor.matmul(
                            accs_of[g][h // 2][:, h % 2, :],
                            lhsT=p_sb[:, tt * QG + h * LQ:
                                      tt * QG + (h + 1) * LQ],
                            rhs=va_of[g][:, t, :],
                            start=(t == first_tile and h % 2 == 0),
                            stop=(t == last_tile),
                            skip_group_check=True,
                        )

            def emit_normalize(g):
                o_sb = ob.tile([128, GROUP * HD], bf16, tag="osb",
                               name=f"osb{g}")
                for p in range(GROUP // 2):
                    acc = accs_of[g][p]
                    rec = rp.tile([LQ, 2], mybir.dt.float32, tag="rec",
                                  name=f"rec_{g}_{p}")
                    nc.vector.reciprocal(rec, acc[:, :, HD:HD + 1])
                    for hh in range(2):
                        h = 2 * p + hh
                        nc.vector.tensor_scalar_mul(
                            o_sb[:, h * HD:(h + 1) * HD],
                            acc[:, hh, 0:HD],
                            rec[:, hh:hh + 1],
                        )
                nc.sync.dma_start(
                    out=o[:, g * GROUP * HD:(g + 1) * GROUP * HD], in_=o_sb)

            work = [(g, ci, t0, w) for g in range(NKV)
                    for ci, (t0, w) in enumerate(chunks)]
            load_group(0, engs=[nc.sync, nc.scalar, nc.sync, nc.scalar,
                                nc.sync])
            nc.scalar.dma_start(out=mask_sb, in_=mk[:, :])
            load_group(1, engs=[nc.scalar, nc.sync, nc.scalar, nc.sync,
                                nc.scalar])
            pending = []  # (g, ci, t0, width, p_sb) awaiting PV emission
            for g, ci, t0, w in work:
                if t0 == 0 and g + 2 < NKV:
                    load_group(g + 2)
                p_sb = emit_qk_exp(g, ci, t0, w)
                pending.append((g, ci, t0, w, p_sb))
                if len(pending) > PIPE:
                    ent = pending.pop(0)
                    emit_pv(*ent)
                    if ent[1] == len(chunks) - 1:
                        emit_normalize(ent[0])
            for ent in pending:
                emit_pv(*ent)
                if ent[1] == len(chunks) - 1:
                    emit_normalize(ent[0])
    return nc


def _prep_inputs(q, k, v, k_cache, v_cache, page_tables, ctx_len, block_size):
    ctx = int(ctx_len)
    bs = int(block_size)
    assert ctx == CTX, f"kernel compiled for ctx_len={CTX}, got {ctx}"
    npages = ctx // PAGE

    q = np.asarray(q, np.float32).reshape(B, LQ, NH, HD)
    k = np.asarray(k, np.float32).reshape(B, LQ, NKV, HD)
    v = np.asarray(v, np.float32).reshape(B, LQ, NKV, HD)
    k_cache = np.asarray(k_cache, np.float32)
    v_cache = np.asarray(v_cache, np.float32)
    pt = np.asarray(page_tables).astype(np.int64)[:, :npages]

    # paged gather: [B, ctx, NKV, HD]
    k_ctx = k_cache[pt].reshape(B, ctx, NKV, HD)
    v_ctx = v_cache[pt].reshape(B, ctx, NKV, HD)
    k_full = np.concatenate([k_ctx, k], axis=1)   # [B, K_TOT, NKV, HD]
    v_full = np.concatenate([v_ctx, v], axis=1)

    # K^T per core: [NKV, HD, K_TOT], bf16
    kt = np.ascontiguousarray(
        k_full.transpose(0, 2, 3, 1)).astype(ml_dtypes.bfloat16)
    # V augmented with ones column, bf16: [B, NKV, K_TOT, HD+1]
    v_t = v_full.transpose(0, 2, 1, 3)            # [B, NKV, K_TOT, HD]
    va = np.empty((B, NKV, K_TOT, HD + 1), np.float32)
    va[..., :HD] = v_t
    va[..., HD] = 1.0
    va = va.astype(ml_dtypes.bfloat16).reshape(B, NKV, KT_TILES, 128, HD + 1)
    # swizzle so each SBUF partition line is one contiguous DMA burst
    va = np.ascontiguousarray(va.transpose(0, 1, 3, 2, 4))
    # Q^T per group: [B, NKV, HD, GROUP*LQ]
    qh = q.transpose(0, 2, 3, 1).reshape(B, NKV, GROUP, HD, LQ)
    qt = np.ascontiguousarray(qh.transpose(0, 1, 3, 2, 4)).reshape(
        B, NKV, HD, QG).astype(ml_dtypes.bfloat16)

    # multiplicative 0/1 mask on the new-token k-tile, scoresT coords [k, q4]
    kj = np.arange(128)
    qi = np.arange(LQ)
    allowed = (kj[:, None] // bs) <= (qi[None, :] // bs)   # [128, LQ]
    mrow = allowed.astype(np.float32)
    mask = np.tile(mrow, (1, GROUP)).astype(ml_dtypes.bfloat16)  # [128, QG]

    in_maps = []
    for b in range(B):
        in_maps.append({
            "qt": qt[b],
            "kt": kt[b],
            "va": va[b],
            "mk": mask,
        })
    return in_maps


def _run(inputs, trace=False):
    from concourse.bass_utils import run_bass_kernel_spmd

    if "nc" not in _CACHE:
        _CACHE["nc"] = _build_nc()
    nc = _CACHE["nc"]
    in_maps = _prep_inputs(**inputs)
    try:
        res = run_bass_kernel_spmd(
            nc, in_maps, core_ids=list(range(B)), trace=trace,
        )
    except Exception:
        # transient NRT device errors have been observed once after heavy
        # compile churn; one retry on a quiesced device is reliable
        import time
        time.sleep(2)
        res = run_bass_kernel_spmd(
            nc, in_maps, core_ids=list(range(B)), trace=trace,
        )
    out = np.empty((B * LQ, NH * HD), np.float32)
    for b in range(B):
        out[b * LQ:(b + 1) * LQ] = np.asarray(
            res.results[b]["o"]).astype(np.float32)
    return out, res


def kernel(**inputs):
    out, _ = _run(inputs, trace=False)
    return out


# revision 23
# speedup vs baseline: 1.2252x; 1.0035x over previous
"""Paged GQA attention (diffusion block-causal) on 8 TRN2 NeuronCores.

Problem: B=8 seqs x LQ=128 new tokens, 32 q heads / 8 kv heads, head_dim 128,
ctx_len=2048 cached tokens per seq (paged KV cache, 16-token pages), plus the
128 new tokens; block-causal mask (block 32) over the new-token region.

Sharding: one sequence per NeuronCore (8 seqs -> 8 cores), no collectives.

Per-core device kernel, per kv-head group g (4 q heads share a kv head):
  scoresT[k, q4] = K_g^T-tile.T @ Q_g          (bf16 matmul, N=512)
  probsT = exp(scoresT * scale)                split across TWO engines:
      ACT chunks:  scalar.activation Exp (exact)
      DVE chunks:  Schraudolph bit-trick: int16(round(s*C + B)) bitcast bf16
                   (C = scale*log2e*128, B = 127*128 + c_corr) ~2-3% elem err
                   on 6/17 tiles -> ~1e-2 output rel err (gate 2e-2)
  probsT[new] *= blockmask                     (Pool/GpSimd, 0/1 multiplicative)
  outU[q, d+1] += probsT_head.T @ [V_g | 1]    (bf16 matmul; last col = sum)
  out[q, d] = outU[:, :d] * (1 / outU[:, d])   (DVE reciprocal + tensor_scalar)

DMA triggers ride the GpSimd queue (cheap: ~25ns vs ~600ns on SP).
Host side: gather pages, build K^T / Q^T / V-augmented layouts, shard, gather.
"""

import sys

if '/opt/trn_rl_repo' not in sys.path:
    sys.path.insert(0, '/opt/trn_rl_repo')

import math

import ml_dtypes
import numpy as np

B = 8
LQ = 128
NH = 32
NKV = 8
GROUP = NH // NKV  # 4
HD = 128
PAGE = 16
CTX = 2048
K_TOT = CTX + LQ          # 2176
KT_TILES = K_TOT // 128   # 17
QG = GROUP * LQ           # 512 (4 heads x 128 queries)
SCALE = 1.0 / math.sqrt(HD)

# ---- tuning flags ----
SCHRAUD_CHUNKS = (1, 3)   # chunk indices exp'd on DVE via Schraudolph
SCHRAUD_C = -8.0          # bias correction (tuned in numpy sim)
MASK_ON_POOL = True       # block-causal mask multiply on GpSimd
PIPE = 3                  # PV trails QK/exp by this many chunks
IN_BUFS = 3               # kt/va/qt tile-pool depth (prefetch 2 groups ahead)
WARM_MMS = 2              # dummy matmuls to ramp the PE p-state during DMA wait

SCH_C7 = SCALE * math.log2(math.e) * 128.0
SCH_B = 127.0 * 128.0 + SCHRAUD_C

_CACHE = {}


def _build_nc():
    import concourse.bass as bass
    import concourse.mybir as mybir
    from concourse.tile import TileContext
    from concourse.vector_clock import ScopedClock

    class TileContextP(TileContext):
        """TileContext adapted to this walrus build, which only supports ONE
        sync-wait per instruction: extra waits are hoisted onto same-engine
        NoOps emitted immediately before the instruction."""

        def _commit_instruction(self, inst, lazy_reg_writes=True):
            si = getattr(inst, "sync_info", None)
            eng = getattr(inst, "engine", None)
            if si is not None and eng is not None:
                waits = list(si.on_wait or [])
                if len(waits) > 1:
                    for w in waits[:-1]:
                        nop = mybir.InstNoOp(
                            name=self.nc.get_next_instruction_name(),
                            sync_info=mybir.SyncInfo(on_wait=[w], on_update=[]),
                            bass_nofuse=True,
                            engine=eng,
                        )
                        super()._commit_instruction(nop, lazy_reg_writes=False)
                    si.on_wait = [waits[-1]]
            return super()._commit_instruction(inst, lazy_reg_writes)

        def _drain_and_barrier(self, tick_clock, wait_clock):
            nc = self.nc
            drain_inst = nc.sync.drain()
            wait_clock.add_sem_waits(
                drain_inst.ins, ScopedClock({None: tick_clock.global_clock})
            )
            si = drain_inst.ins.sync_info
            waits = list(si.on_wait or []) if si is not None else []
            if len(waits) > 1:
                si.on_wait = [waits[0]]
                # distribute remaining waits round-robin across engines so
                # they resolve in parallel rather than serially on SP
                engs = [nc.vector, nc.scalar, nc.tensor, nc.gpsimd, nc.sync]
                for j, w in enumerate(waits[1:]):
                    d = engs[j % len(engs)].drain()
                    d.ins.sync_info = mybir.SyncInfo(on_wait=[w], on_update=[])
            nc.all_engine_barrier(sem_only=True)
            assert self.sems is not None
            popped = nc._tile_sem_poison_stack.pop()
            assert popped is self._sem_poison
            nc.clear_and_free_semaphores(list(self.sems.allocated().values()))
            nc.all_engine_barrier(sem_only=True)

    f32 = mybir.dt.float32
    bf16 = mybir.dt.bfloat16
    i16 = mybir.dt.int16

    nc = bass.Bass("TRN2")
    qt = nc.dram_tensor("qt", [NKV, HD, QG], bf16, kind="ExternalInput")
    kt = nc.dram_tensor("kt", [NKV, HD, K_TOT], bf16, kind="ExternalInput")
    va = nc.dram_tensor("va", [NKV, 128, KT_TILES, HD + 1], bf16,
                        kind="ExternalInput")
    mk = nc.dram_tensor("mk", [128, QG], bf16, kind="ExternalInput")
    o = nc.dram_tensor("o", [LQ, NH * HD], bf16, kind="ExternalOutput")

    # k-tiles are exp'd in triples: one [128, 3*QG] PSUM chunk spans three
    # banks, cutting the per-instruction fixed cost. 17 tiles -> 5 triples +
    # 1 pair. PSUM: scores 3 banks x 2 bufs + 2 packed accumulator banks = 8.
    chunks = [(3 * i, 3) for i in range(5)] + [(15, 2)]
    first_tile, last_tile = 0, KT_TILES - 1

    with TileContextP(nc) as tc:
        maske = nc.gpsimd if MASK_ON_POOL else nc.vector
        with (
            tc.tile_pool(name="kp", bufs=IN_BUFS) as kp,
            tc.tile_pool(name="vp", bufs=IN_BUFS) as vp,
            tc.tile_pool(name="qp", bufs=IN_BUFS) as qp,
            tc.tile_pool(name="mp", bufs=1) as mp,
            tc.tile_pool(name="pp", bufs=5) as pp,
            tc.tile_pool(name="rp", bufs=4) as rp,
            tc.tile_pool(name="ob", bufs=2) as ob,
            tc.tile_pool(name="sp", bufs=2, space="PSUM") as sp,
            tc.tile_pool(name="op", bufs=2, space="PSUM") as op,
        ):
            mask_sb = mp.tile([128, QG], bf16)

            # Flat software pipeline over (group, chunk): each chunk's PV
            # matmuls are emitted AFTER later chunks' QK matmuls so the
            # in-order PE stream never head-of-line blocks on exp results or
            # on the previous group's accumulator release.
            accs_of = {}
            qt_of, kt_of, va_of = {}, {}, {}

            def load_group(g, engs=None):
                # hardware-DGE trigger engines; prologue groups parallelize
                # across idle queues, steady state rides SP (625ns/trigger)
                if engs is None:
                    engs = [nc.sync] * 5
                kt_sb = kp.tile([HD, K_TOT], bf16, tag="kt", name=f"kt{g}")
                # split so the first chunks' K columns land early; kt leads
                # qt since LDWEIGHTS consumes it first
                engs[1].dma_start(out=kt_sb[:, :3 * 128],
                                  in_=kt[g][:, :3 * 128])
                qt_sb = qp.tile([HD, QG], bf16, tag="qt", name=f"qt{g}")
                engs[0].dma_start(out=qt_sb, in_=qt[g])
                engs[2].dma_start(out=kt_sb[:, 3 * 128:6 * 128],
                                  in_=kt[g][:, 3 * 128:6 * 128])
                engs[3].dma_start(out=kt_sb[:, 6 * 128:],
                                  in_=kt[g][:, 6 * 128:])
                va_sb = vp.tile([128, KT_TILES, HD + 1], bf16, tag="va",
                                name=f"va{g}")
                engs[4].dma_start(out=va_sb, in_=va[g])
                kt_of[g], va_of[g], qt_of[g] = kt_sb, va_sb, qt_sb
                # two heads' [q, d+1] accumulators packed per PSUM bank
                accs_of[g] = [
                    op.tile([LQ, 2, HD + 1], mybir.dt.float32, tag="acc",
                            name=f"acc_{g}_{p}")
                    for p in range(GROUP // 2)
                ]

            def emit_qk_exp(g, ci, t0, width):
                s_ps = sp.tile([128, 3 * QG], mybir.dt.float32, tag="s",
                               name=f"s_{g}_{t0}")
                for tt in range(width):
                    nc.tensor.matmul(
                        s_ps[:, tt * QG:(tt + 1) * QG],
                        lhsT=kt_of[g][:, (t0 + tt) * 128:(t0 + tt + 1) * 128],
                        rhs=qt_of[g],
                        start=True,
                        stop=True,
                    )
                p_sb = pp.tile([128, 3 * QG], bf16, tag="p",
                               name=f"p_{g}_{t0}")
                if ci in SCHRAUD_CHUNKS:
                    # Schraudolph fast-exp on DVE: the int16 affine result IS
                    # the bf16 bit pattern of ~exp(s*scale)
                    nc.vector.tensor_scalar(
                        p_sb[:, :width * QG].bitcast(i16),
                        s_ps[:, :width * QG],
                        SCH_C7, SCH_B,
                        mybir.AluOpType.mult, mybir.AluOpType.add,
                    )
                else:
                    nc.scalar.activation(
                        p_sb[:, :width * QG], s_ps[:, :width * QG],
                        mybir.ActivationFunctionType.Exp, scale=SCALE,
                    )
                if t0 + width == KT_TILES:
                    # new-token tile: multiplicative block-causal mask applied
                    # post-exp (keeps ACT's critical path mask-free)
                    lo = (width - 1) * QG
                    maske.tensor_mul(
                        p_sb[:, lo:lo + QG], p_sb[:, lo:lo + QG], mask_sb)
                return p_sb

            def emit_pv(g, ci, t0, width, p_sb):
                for tt in range(width):
                    t = t0 + tt
                    for h in range(GROUP):
                        # start=True clears has_written for the WHOLE bank, so
                        # only the first head sharing the bank issues it; the
                        # second head's first-tile write lands on
                        # has_written=0 and overwrites rather than accumulates.
                        nc.tensor.matmul(
                            accs_of[g][h // 2][:, h % 2, :],
                            lhsT=p_sb[:, tt * QG + h * LQ:
                                      tt * QG + (h + 1) * LQ],
                            rhs=va_of[g][:, t, :],
                            start=(t == first_tile and h % 2 == 0),
                            stop=(t == last_tile),
                            skip_group_check=True,
                        )

            def emit_normalize(g):
                o_sb = ob.tile([128, GROUP * HD], bf16, tag="osb",
                               name=f"osb{g}")
                half = 2 * HD
                for p in range(GROUP // 2):
                    acc = accs_of[g][p]
                    rec = rp.tile([LQ, 2], mybir.dt.float32, tag="rec",
                                  name=f"rec_{g}_{p}")
                    nc.vector.reciprocal(rec, acc[:, :, HD:HD + 1])
                    for hh in range(2):
                        h = 2 * p + hh
                        nc.vector.tensor_scalar_mul(
                            o_sb[:, h * HD:(h + 1) * HD],
                            acc[:, hh, 0:HD],
                            rec[:, hh:hh + 1],
                        )
                    # per-bank halves: the first half's DMA overlaps the
                    # second bank's normalize (matters for the last group)
                    nc.sync.dma_start(
                        out=o[:, g * GROUP * HD + p * half:
                              g * GROUP * HD + (p + 1) * half],
                        in_=o_sb[:, p * half:(p + 1) * half])

            work = [(g, ci, t0, w) for g in range(NKV)
                    for ci, (t0, w) in enumerate(chunks)]
            # group-0 triggers lead everything: scalar's queue drains its
            # preamble first, so the critical kt chunk + qt ride scalar
            load_group(0, engs=[nc.scalar, nc.scalar, nc.sync, nc.sync,
                                nc.sync])
            nc.scalar.dma_start(out=mask_sb, in_=mk[:, :])
            load_group(1, engs=[nc.scalar, nc.sync, nc.scalar, nc.sync,
                                nc.scalar])

            # warm the ACT exp table while the first DMAs are in flight
            warm = mp.tile([128, 1], mybir.dt.float32, name="warm")
            nc.scalar.memzero(warm)
            nc.scalar.activation(warm, warm,
                                 mybir.ActivationFunctionType.Exp)

            # ramp the PE p-state (cold: 0.65 GHz -> 2.4 GHz after ~3us of
            # work) with dummy matmuls on zeroed SBUF while DMAs land
            if WARM_MMS:
                z_l = mp.tile([128, 128], bf16, name="warm_l")
                z_r = mp.tile([128, QG], bf16, name="warm_r")
                nc.gpsimd.memzero(z_l)
                nc.gpsimd.memzero(z_r)
                warm_ps = sp.tile([128, 3 * QG], mybir.dt.float32, tag="s",
                                  name="warm_ps")
                for _ in range(WARM_MMS):
                    nc.tensor.matmul(warm_ps[:, :QG], lhsT=z_l, rhs=z_r,
                                     start=True, stop=True)
            pending = []  # (g, ci, t0, width, p_sb) awaiting PV emission
            for g, ci, t0, w in work:
                if t0 == 0 and g + 2 < NKV:
                    load_group(g + 2)
                p_sb = emit_qk_exp(g, ci, t0, w)
                pending.append((g, ci, t0, w, p_sb))
                if len(pending) > PIPE:
                    ent = pending.pop(0)
                    emit_pv(*ent)
                    if ent[1] == len(chunks) - 1:
                        emit_normalize(ent[0])
            for ent in pending:
                emit_pv(*ent)
                if ent[1] == len(chunks) - 1:
                    emit_normalize(ent[0])
    return nc


def _prep_inputs(q, k, v, k_cache, v_cache, page_tables, ctx_len, block_size):
    ctx = int(ctx_len)
    bs = int(block_size)
    assert ctx == CTX, f"kernel compiled for ctx_len={CTX}, got {ctx}"
    npages = ctx // PAGE

    q = np.asarray(q, np.float32).reshape(B, LQ, NH, HD)
    k = np.asarray(k, np.float32).reshape(B, LQ, NKV, HD)
    v = np.asarray(v, np.float32).reshape(B, LQ, NKV, HD)
    k_cache = np.asarray(k_cache, np.float32)
    v_cache = np.asarray(v_cache, np.float32)
    pt = np.asarray(page_tables).astype(np.int64)[:, :npages]

    # paged gather: [B, ctx, NKV, HD]
    k_ctx = k_cache[pt].reshape(B, ctx, NKV, HD)
    v_ctx = v_cache[pt].reshape(B, ctx, NKV, HD)
    k_full = np.concatenate([k_ctx, k], axis=1)   # [B, K_TOT, NKV, HD]
    v_full = np.concatenate([v_ctx, v], axis=1)

    # K^T per core: [NKV, HD, K_TOT], bf16
    kt = np.ascontiguousarray(
        k_full.transpose(0, 2, 3, 1)).astype(ml_dtypes.bfloat16)
    # V augmented with ones column, bf16: [B, NKV, K_TOT, HD+1]
    v_t = v_full.transpose(0, 2, 1, 3)            # [B, NKV, K_TOT, HD]
    va = np.empty((B, NKV, K_TOT, HD + 1), np.float32)
    va[..., :HD] = v_t
    va[..., HD] = 1.0
    va = va.astype(ml_dtypes.bfloat16).reshape(B, NKV, KT_TILES, 128, HD + 1)
    # swizzle so each SBUF partition line is one contiguous DMA burst
    va = np.ascontiguousarray(va.transpose(0, 1, 3, 2, 4))
    # Q^T per group: [B, NKV, HD, GROUP*LQ]
    qh = q.transpose(0, 2, 3, 1).reshape(B, NKV, GROUP, HD, LQ)
    qt = np.ascontiguousarray(qh.transpose(0, 1, 3, 2, 4)).reshape(
        B, NKV, HD, QG).astype(ml_dtypes.bfloat16)

    # multiplicative 0/1 mask on the new-token k-tile, scoresT coords [k, q4]
    kj = np.arange(128)
    qi = np.arange(LQ)
    allowed = (kj[:, None] // bs) <= (qi[None, :] // bs)   # [128, LQ]
    mrow = allowed.astype(np.float32)
    mask = np.tile(mrow, (1, GROUP)).astype(ml_dtypes.bfloat16)  # [128, QG]

    in_maps = []
    for b in range(B):
        in_maps.append({
            "qt": qt[b],
            "kt": kt[b],
            "va": va[b],
            "mk": mask,
        })
    return in_maps


def _run(inputs, trace=False):
    from concourse.bass_utils import run_bass_kernel_spmd

    if "nc" not in _CACHE:
        _CACHE["nc"] = _build_nc()
    nc = _CACHE["nc"]
    in_maps = _prep_inputs(**inputs)
    try:
        res = run_bass_kernel_spmd(
            nc, in_maps, core_ids=list(range(B)), trace=trace,
        )
    except Exception:
        # transient NRT device errors have been observed once after heavy
        # compile churn; one retry on a quiesced device is reliable
        import time
        time.sleep(2)
        res = run_bass_kernel_spmd(
            nc, in_maps, core_ids=list(range(B)), trace=trace,
        )
    out = np.empty((B * LQ, NH * HD), np.float32)
    for b in range(B):
        out[b * LQ:(b + 1) * LQ] = np.asarray(
            res.results[b]["o"]).astype(np.float32)
    return out, res


def kernel(**inputs):
    out, _ = _run(inputs, trace=False)
    return out
